# revision 8
# baseline (speedup 1.0000x reference)
"""Trainium2 Bass kernel for nn_BoxCrossAttention_352187318473.

Math: the reference's attention has a single KV token, so the softmax over
the key axis (length 1) is exactly 1.0 and the output is independent of
x / Wp / Wq / Wk.  The whole module collapses to

    o   = ((mish(y @ W1 + b1) @ W2 + b2)[:, KV:] @ Wv + bv) @ Wo + bo
    out[b, c, w, h] = 9 * o[b, c]          (9 = kernel_size**2 positions)

The three trailing linear maps have no nonlinearity between them, so the
host constant-folds the weights (weight-only preprocessing, exact f32),
including the x9 spatial factor:

    Wfold9 = 9 * W2[:, KV:] @ Wv @ Wo               [1024, 256]
    cb9    = 9 * (b2[KV:] @ Wv @ Wo + bv @ Wo + bo) [256]
    out[b, :, w, h] = mish(y_b @ W1 + b1) @ Wfold9 + cb9

All data-dependent compute (everything touching y) runs on device.

Sharding: output viewed as [B*C, W*H] = [1024, 4096]; core i produces rows
[i*128, (i+1)*128) = batch i//2, channel half i%2.  Per-core device work:
  - one fp16 pack [128, 3084]: y(2) | b1(8) | cb9 hi/lo(2) | W1 (m-groups,
    k-within) | Wfold9-slice, streamed as 4 DMAs so compute trails the
    stream;
  - L1 into PSUM, then a single mish chain using one activation table
    (Exp only; table load hidden under the load phase):
        mish(v) = v*a/(a+2),  a = e*(e+2),  e = exp(v)
    via one Exp + 6 DVE ops (incl. the native DVE reciprocal);
  - L2 accumulates o9 = 9*o per channel half (PSUM partitions 0:64/64:128);
  - bc[rows] = cbb[rows] + ps_o (one tensor_scalar per half; cbb is the
    cb9 bias pre-broadcast to 256 cols during the load phase);
  - the store DMA replicates bc's 256 fp16 cols 16x via a stride-0 outer
    dim (fastest dim contiguous, full DMA bandwidth), fp16 output (~5e-4
    rounding), host upcasts while unsharding.
"""

import numpy as np

import concourse.bacc as bacc
import concourse.tile as tile
from concourse import mybir
from concourse.bass_utils import run_bass_kernel_spmd

F32 = mybir.dt.float32
F16 = mybir.dt.float16
AF = mybir.ActivationFunctionType
ALU = mybir.AluOpType

B, C, W, H = 4, 256, 64, 64
WH = W * H            # 4096
TAU = 256
KV = 512
N_CORES = 8

# fp16 pack layout: y(2) | b1(8) | cb9 hi/lo(2) | W1 (2 m-groups x 1024) |
# Wfold9 (2 halves x 512)
OFF_Y = 0
OFF_B1 = 2
OFF_CB = 10
OFF_W1 = 12
OFF_WF = OFF_W1 + 2048
PK_W = OFF_WF + 1024

BC_W = 256            # materialized broadcast cols; store replicates 16x

OUT_DT = F16

_nc_cache = None


def _build_nc():
    nc = bacc.Bacc(trn_type="TRN2")

    pk = nc.dram_tensor("pk", [128, PK_W], F16, kind="ExternalInput")
    outd = nc.dram_tensor("out", [128, WH], OUT_DT, kind="ExternalOutput")

    with tile.TileContext(nc) as tc:
        with (
            tc.tile_pool(name="wp", bufs=1) as wp,
            tc.tile_pool(name="ap", bufs=1) as ap,
            tc.tile_pool(name="pp", bufs=1, space="PSUM") as pp,
        ):
            p = wp.tile([128, PK_W], F16, tag="p")
            # 4 sub-DMAs of one pack: y/b1/cb9/W1-group0, W1-group1, Wf h0, Wf h1
            nc.sync.dma_start(out=p[:, 0:OFF_W1 + 1024], in_=pk[:, 0:OFF_W1 + 1024])
            nc.sync.dma_start(out=p[:, OFF_W1 + 1024:OFF_WF],
                              in_=pk[:, OFF_W1 + 1024:OFF_WF])
            nc.sync.dma_start(out=p[:, OFF_WF:OFF_WF + 512],
                              in_=pk[:, OFF_WF:OFF_WF + 512])
            nc.sync.dma_start(out=p[:, OFF_WF + 512:PK_W],
                              in_=pk[:, OFF_WF + 512:PK_W])

            y_sb = p[:, OFF_Y:OFF_Y + 2]

            def w1(g, m, k):            # W1 lhsT chunk: m-group g, m in 0..3, k in 0..1
                off = OFF_W1 + 1024 * g + 256 * m + 128 * k
                return p[:, off:off + 128]

            def wf(h, k):               # Wfold9 (half h, k-chunk): [128, 64]
                return p[:, OFF_WF + 512 * h + 64 * k: OFF_WF + 512 * h + 64 * k + 64]

            # off-path prep from D1: b1 -> f32, cb9 hi+lo -> f32 -> broadcast 256
            b1f = ap.tile([128, 8], F32, tag="b1f")
            nc.vector.tensor_copy(out=b1f, in_=p[:, OFF_B1:OFF_B1 + 8])
            cb9f = ap.tile([128, 1], F32, tag="cb9f")
            nc.vector.tensor_tensor(out=cb9f, in0=p[:, OFF_CB:OFF_CB + 1],
                                    in1=p[:, OFF_CB + 1:OFF_CB + 2], op=ALU.add)
            cbb = ap.tile([128, BC_W], F16, tag="cbb")
            nc.vector.tensor_scalar(out=cbb, in0=p[:, OFF_B1:OFF_B1 + 2 + BC_W - 2],
                                    scalar1=0.0, scalar2=cb9f[:, 0:1],
                                    op0=ALU.mult, op1=ALU.add)

            # ---- L1 + mish (single chain; all-DVE after one Exp) ----
            ps_t1 = pp.tile([128, 8], F32, tag="ps_t1")
            for g in range(2):
                for m in range(4):
                    for k in range(2):
                        nc.tensor.matmul(
                            out=ps_t1[:, 4 * g + m:4 * g + m + 1],
                            lhsT=w1(g, m, k),
                            rhs=y_sb[:, k:k + 1],
                            start=(k == 0),
                            stop=(k == 1),
                        )
            # mish(v) = v*a/(a+2), a = e*(e+2), e = exp(v)
            v = ap.tile([128, 8], F32, tag="v")
            nc.vector.tensor_add(out=v, in0=ps_t1, in1=b1f)
            e = ap.tile([128, 8], F32, tag="e")
            nc.scalar.activation(out=e, in_=v, func=AF.Exp)
            ep2 = ap.tile([128, 8], F32, tag="ep2")
            nc.vector.tensor_scalar(out=ep2, in0=e, scalar1=2.0, scalar2=None,
                                    op0=ALU.add)
            a = ap.tile([128, 8], F32, tag="a")
            nc.vector.tensor_mul(out=a, in0=e, in1=ep2)
            num = ap.tile([128, 8], F32, tag="num")
            nc.vector.tensor_mul(out=num, in0=v, in1=a)
            den = ap.tile([128, 8], F32, tag="den")
            nc.vector.tensor_scalar(out=den, in0=a, scalar1=2.0, scalar2=None,
                                    op0=ALU.add)
            rinv = ap.tile([128, 8], F32, tag="rinv")
            nc.vector.reciprocal(out=rinv, in_=den)
            m1 = ap.tile([128, 8], F16, tag="m1")
            nc.vector.tensor_mul(out=m1, in0=num, in1=rinv)

            # ---- L2 (o9 in PSUM, per channel half) + broadcast + store ----
            ps_o = pp.tile([128, 1], F32, tag="ps_o")
            bc = ap.tile([128, BC_W], F16, tag="bc")
            for h in range(2):
                rows = slice(64 * h, 64 * h + 64)
                for k in range(8):
                    nc.tensor.matmul(
                        out=ps_o[rows, 0:1], lhsT=wf(h, k), rhs=m1[:, k:k + 1],
                        start=(k == 0), stop=(k == 7),
                    )
                nc.vector.tensor_scalar(
                    out=bc[rows, :], in0=cbb[rows, :],
                    scalar1=0.0, scalar2=ps_o[rows, 0:1],
                    op0=ALU.add, op1=ALU.add,
                )
                rep = bc[rows, :].unsqueeze(1).to_broadcast([64, WH // BC_W, BC_W])
                nc.sync.dma_start(out=outd[rows, :], in_=rep)

    return nc


def _host_in_maps(y, W1, b1, W2, b2, Wv, bv, Wo, bo):
    # weight-only constant folding of the three trailing linear maps and the
    # x9 spatial factor (f32)
    WvWo = Wv @ Wo                                           # [KV, C]
    Wfold9 = 9.0 * (W2[:, KV:] @ WvWo)                       # [2*KV, C]
    cb9 = 9.0 * (b2[KV:] @ WvWo + bv @ Wo + bo)              # [C]

    # W1 packed as 2 m-groups; within a group: m-chunk-major, k-within:
    # col 1024*g + 256*m + 128*k + j  <->  W1[128k+p, 512g + 128m + j]
    w1p = (W1.reshape(2, 128, 2, 4, 128)     # [k, kp, g, m, j]
           .transpose(1, 2, 3, 0, 4)         # [kp, g, m, k, j]
           .reshape(128, 2048).astype(np.float16))

    in_maps = []
    for core in range(N_CORES):
        b_i, half = core // 2, core % 2
        ch = slice(half * 128, (half + 1) * 128)
        cbs = cb9[ch]
        cb_hi = cbs.astype(np.float16)
        cb_lo = (cbs - cb_hi.astype(np.float32)).astype(np.float16)
        # Wfold9 slice packed per (out-half h, k-chunk c): rows 128c..,
        # cols 64h..64h+64
        wfs = Wfold9[:, ch].reshape(8, 128, 2, 64)           # [kc, kp, h, m]
        wfp = wfs.transpose(1, 2, 0, 3).reshape(128, 1024).astype(np.float16)

        pk = np.empty((128, PK_W), np.float16)
        pk[:, OFF_Y:OFF_Y + 2] = y[b_i].reshape(2, 128).T.astype(np.float16)
        pk[:, OFF_B1:OFF_B1 + 8] = b1.reshape(8, 128).T.astype(np.float16)
        pk[:, OFF_CB] = cb_hi
        pk[:, OFF_CB + 1] = cb_lo
        pk[:, OFF_W1:OFF_W1 + 2048] = w1p
        pk[:, OFF_WF:PK_W] = wfp
        in_maps.append({"pk": pk})
    return in_maps


def run(inputs, trace=False, **kw):
    global _nc_cache
    if _nc_cache is None:
        _nc_cache = _build_nc()
        _nc_cache.finalize()
    nc = _nc_cache
    in_maps = _host_in_maps(
        np.asarray(inputs["y"], np.float32),
        np.asarray(inputs["W1"], np.float32), np.asarray(inputs["b1"], np.float32),
        np.asarray(inputs["W2"], np.float32), np.asarray(inputs["b2"], np.float32),
        np.asarray(inputs["Wv"], np.float32), np.asarray(inputs["bv"], np.float32),
        np.asarray(inputs["Wo"], np.float32), np.asarray(inputs["bo"], np.float32),
    )
    res = run_bass_kernel_spmd(nc, in_maps, core_ids=list(range(N_CORES)),
                               trace=trace, **kw)
    flat = np.empty((B * C, WH), np.float32)
    for core in range(N_CORES):
        flat[core * 128:(core + 1) * 128] = res.results[core]["out"].astype(np.float32)
    out = flat.reshape(B, C, W, H)
    return out, res


def kernel(**inputs):
    out, _ = run(inputs, trace=False)
    return out
